# revision 1
# baseline (speedup 1.0000x reference)
# Trainium2 Bass kernel for nn_LogitsNew (dense_mlp).
#
#   u = gelu(x @ W_proj + b_proj)                       [B, D]
#   logits = (u @ W_u)[:, None, :] + ee @ W_e           [B, N, C]
#
# Sharding: data-parallel over batch B across 8 cores (4 batches/core).
#
# All data moves as bf16 (host-cast; ~0.3% norm rel err, gate is 2e-2):
# halves HBM traffic vs fp32 to ~10MB/core. ee is transposed on the host
# into k-slice-major lhsT layout, eliminating all on-device PE transposes
# of ee. Output is stored bf16 and upcast on the host.
#
# The kernel is PE-stream-bound (~200 N=512 matmuls at ~220ns warm), so
# the structure keeps the PE issuing back-to-back and pushes everything
# else off the critical path:
#   - warmup/filler dummies keep the HAM clock-gate at 2.4GHz (a >3.4us
#     idle re-throttles the PE to 1.2GHz).
#   - phase 1: k-outer accumulation over m-tiles 0..3 (8 PSUM banks),
#     consuming eet_lo + W_e k-slices as they stream in on two rings.
#   - utterance path SECOND (W_proj arrives ~24us): z = x@W_proj + b
#     (K=1 ones matmul bias), u = Gelu(z) on ACT, uT via PE transposes
#     into ONE psum bank, padded to 33 cols so y lands with batch 2 at
#     partition 0 and batch 3 at partition 32 (the only legal K=1 rhs
#     base partitions), y = uT.T@W_u.
#   - m-tiles 4..7 LAST, each closed by an in-PSUM K=1 y-add matmul ->
#     drains are plain copies, stores stream out, tail is one tile.
#   - m0..m3 get y added in an all-bf16 overlapped epilogue on DVE
#     (in-place, 2x DVE rate) while the PE crunches m4..m7.
#   - drains split between DVE (h0) and ACT (h1); stores split across
#     both HWDGE rings.

import sys

if "/opt/trn_rl_repo" not in sys.path:
    sys.path.insert(0, "/opt/trn_rl_repo")

import numpy as np
import ml_dtypes

import concourse.bass as bass
import concourse.mybir as mybir
import concourse.tile as tile
from concourse import bacc
from concourse.bass_utils import run_bass_kernel_spmd
from concourse.masks import make_identity

P = 128
B, N, D, C = 32, 256, 1024, 1024
NCORES = 8
BPC = B // NCORES          # batches per core
KT = D // P                # 8 k-tiles over the contraction dim
FD = 512                   # matmul moving free dim (one PSUM bank of fp32)
NT = N // P                # 2 n-tiles per batch
MT = BPC * NT              # 8 m-tiles per core
HM = MT // 2 * P           # m-column split point (512)
UP = 33                    # padded uT columns (batch 3 at col 32)

F32 = mybir.dt.float32
BF16 = mybir.dt.bfloat16
GELU = mybir.ActivationFunctionType.Gelu
BF = ml_dtypes.bfloat16

# batch -> y partition: b2 -> 0, b0 -> 1, b1 -> 2, b3 -> 32
YPART = {0: 1, 1: 2, 2: 0, 3: 32}

_CACHE = {}


def _build():
    if "nc" in _CACHE:
        return _CACHE["nc"]

    nc = bacc.Bacc("TRN2", target_bir_lowering=False, debug=False, num_devices=NCORES)

    # host-packed inputs (see kernel() for the packing)
    eet = nc.dram_tensor("eet", [KT, P, MT * P], BF16, kind="ExternalInput").ap()
    we = nc.dram_tensor("we", [KT, P, C], BF16, kind="ExternalInput").ap()
    wp = nc.dram_tensor("wp", [KT, P, C], BF16, kind="ExternalInput").ap()
    wu = nc.dram_tensor("wu", [KT, P, C], BF16, kind="ExternalInput").ap()
    xt = nc.dram_tensor("xt", [P, KT, BPC], BF16, kind="ExternalInput").ap()
    bp = nc.dram_tensor("bp", [1, D], BF16, kind="ExternalInput").ap()
    out = nc.dram_tensor("logits", [MT, P, C], BF16, kind="ExternalOutput").ap()

    with tile.TileContext(nc) as tc:
        with (
            tc.tile_pool(name="const", bufs=1) as cpool,
            tc.tile_pool(name="weights", bufs=1) as wpool,
            tc.tile_pool(name="opre", bufs=1) as oprepool,
            tc.tile_pool(name="obf", bufs=1) as obfpool,
            tc.tile_pool(name="mm_ps", bufs=8, space="PSUM") as mm_ps,
        ):
            we_sb = wpool.tile([P, KT, C], BF16)
            eet_sb = wpool.tile([P, KT, MT * P], BF16)
            wp_sb = wpool.tile([P, KT, C], BF16)
            wu_sb = wpool.tile([P, KT, C], BF16)
            xt_sb = cpool.tile([P, KT, BPC], BF16)
            b_sb = cpool.tile([1, D], BF16)

            # sync/SP ring: first eet_lo pair, W_e pairs, then W_u
            nc.sync.dma_start(
                eet_sb[:, 0:2, 0:HM], eet[0:2, :, 0:HM].rearrange("a p m -> p a m")
            )
            for j in range(KT // 2):
                ks = slice(2 * j, 2 * j + 2)
                nc.sync.dma_start(
                    we_sb[:, ks, :], we[ks].rearrange("a p c -> p a c")
                )
            for j in range(2):
                ks = slice(4 * j, 4 * j + 4)
                nc.sync.dma_start(
                    wu_sb[:, ks, :], wu[ks].rearrange("a p c -> p a c")
                )
            # scalar/ACT ring: eet_lo pairs 1-3, x/b, W_proj, eet_hi
            for j in range(1, KT // 2):
                ks = slice(2 * j, 2 * j + 2)
                nc.scalar.dma_start(
                    eet_sb[:, ks, 0:HM],
                    eet[ks, :, 0:HM].rearrange("a p m -> p a m"),
                )
            nc.scalar.dma_start(xt_sb, xt)
            nc.scalar.dma_start(b_sb, bp)
            for j in range(2):
                ks = slice(4 * j, 4 * j + 4)
                nc.scalar.dma_start(
                    wp_sb[:, ks, :], wp[ks].rearrange("a p c -> p a c")
                )
            for j in range(KT // 2):
                ks = slice(2 * j, 2 * j + 2)
                nc.scalar.dma_start(
                    eet_sb[:, ks, HM:],
                    eet[ks, :, HM:].rearrange("a p m -> p a m"),
                )

            # ---- constants ----
            ones_big = cpool.tile([P, FD], BF16)
            nc.gpsimd.memset(ones_big, 1.0)
            ident_f = cpool.tile([P, P], F32)
            make_identity(nc, ident_f)

            # ---- PSUM allocation (8-slot ring; order matches bank
            # lifetimes) ----
            dummy = mm_ps.tile([P, FD], F32, tag="mm", name="dummy")   # s0
            ph1 = {}
            for m in range(4):
                for h in range(2):                                     # s1-s7,s0
                    ph1[m, h] = mm_ps.tile([P, FD], F32, tag="mm", name=f"p1_{m}_{h}")
            zps = [
                mm_ps.tile([P, FD], F32, tag="mm", name=f"z_{h}")      # s1,s2
                for h in range(2)
            ]
            tp_all = mm_ps.tile([P, FD], F32, tag="mm", name="tp_all")  # s3
            yps = [
                mm_ps.tile([P, FD], F32, tag="mm", name=f"y_{h}")      # s4,s5
                for h in range(2)
            ]
            mp = {}
            for mt in (4, 5, 6, 7):                                    # s6,s7,s0..s5
                mp[mt] = [
                    mm_ps.tile([P, FD], F32, tag="mm", name=f"p3_{mt}_{h}")
                    for h in range(2)
                ]

            def filler(n):
                # HAM keep-warm: harmless matmuls during sub-us stalls
                for _ in range(n):
                    nc.tensor.matmul(
                        dummy[:, :256], ones_big[:, :P], ones_big[:, :256],
                        start=True, stop=True,
                    )

            # ---- PE warmup ----
            filler(14)

            def drain(o, mps):
                # split the two PSUM halves across DVE and ACT
                nc.vector.tensor_copy(o[:, 0:FD], mps[0])
                nc.scalar.copy(o[:, FD:C], mps[1])

            # ---- phase 1: k-outer accumulation over m-tiles 0..3 ----
            for ko in range(KT):
                for m in range(4):
                    ms = slice(m * P, (m + 1) * P)
                    for h in range(2):
                        nc.tensor.matmul(
                            ph1[m, h],
                            eet_sb[:, ko, ms],
                            we_sb[:, ko, h * FD : (h + 1) * FD],
                            start=(ko == 0),
                            stop=(ko == KT - 1),
                        )
                if ko in (0, 1, 2):
                    filler(3)  # keep HAM busy across the supply gap
            opre = {}
            for m in range(4):
                opre[m] = oprepool.tile([P, C], BF16, tag=f"f{m}", name=f"opre_{m}")
                drain(opre[m], [ph1[m, 0], ph1[m, 1]])

            # ---- utterance path ----
            filler(2)  # cover the W_proj arrival edge
            # z = x @ W_proj + b, via K=1 ones matmul for the bias
            u32 = cpool.tile([BPC, C], F32)
            for h in range(2):
                cs = slice(h * FD, (h + 1) * FD)
                for ko in range(KT):
                    nc.tensor.matmul(
                        zps[h][:BPC], xt_sb[:, ko, :], wp_sb[:, ko, cs],
                        start=(ko == 0), stop=False,
                    )
                nc.tensor.matmul(
                    zps[h][:BPC], ones_big[:1, :BPC], b_sb[:1, cs],
                    start=False, stop=True,
                )
            for h in range(2):
                cs = slice(h * FD, (h + 1) * FD)
                nc.scalar.activation(u32[:, cs], zps[h][:BPC], GELU)

            # uT via PE transposes into ONE psum bank; padded copy-out so
            # y rows land at partitions {0:b2, 1:b0, 2:b1, 32:b3}
            uT = cpool.tile([P, KT, UP], BF16)
            for ko in range(KT):
                nc.tensor.transpose(
                    tp_all[:, ko * BPC : (ko + 1) * BPC],
                    u32[:BPC, ko * P : (ko + 1) * P],
                    ident_f[:BPC, :BPC],
                )
            filler(3)  # cover the uT copy-out latency
            tpv = tp_all[:, : KT * BPC].rearrange("p (a b) -> p a b", b=BPC)
            nc.vector.tensor_copy(uT[:, :, 0], tpv[:, :, 2])
            nc.scalar.copy(uT[:, :, 1:3], tpv[:, :, 0:2])
            nc.vector.tensor_copy(uT[:, :, 32], tpv[:, :, 3])

            # y = u @ W_u  ->  [33, C] psum -> bf16
            y_bf = cpool.tile([UP, C], BF16)
            for h in range(2):
                cs = slice(h * FD, (h + 1) * FD)
                for ko in range(KT):
                    nc.tensor.matmul(
                        yps[h][:UP], uT[:, ko, :], wu_sb[:, ko, cs],
                        start=(ko == 0), stop=(ko == KT - 1),
                    )
                nc.scalar.copy(y_bf[:, cs], yps[h][:UP])
            filler(3)  # cover the y_bf copy latency

            # epilogue machinery for m0..m3 (all bf16, off critical path):
            # broadcast y rows for batches 0/1, add in-place, store on ACT
            y_row = cpool.tile([1, 2, C], BF16)
            nc.scalar.dma_start(y_row, y_bf[1:3, :])
            ybc = cpool.tile([P, 2, C], BF16)
            for b2 in (0, 1):
                nc.gpsimd.partition_broadcast(ybc[:, b2, :], y_row[:1, b2, :])
            for m in range(4):
                nc.vector.tensor_add(opre[m], opre[m], ybc[:, m // NT, :])
                nc.scalar.dma_start(out[m], opre[m])

            # ---- m-tiles 4..7: k-inner + in-PSUM y-add, plain drains.
            # h-sequential with per-half drains + stores so the h0 drain
            # and store overlap the h1 matmuls (shortens the m7 tail) ----
            obf = {}
            for mt in (4, 5, 6, 7):
                ms = slice(mt * P, (mt + 1) * P)
                yp_row = YPART[mt // NT]
                o = obfpool.tile([P, C], BF16, tag=f"o{mt}", name=f"obf_{mt}")
                obf[mt] = o
                for h in range(2):
                    cs = slice(h * FD, (h + 1) * FD)
                    for ko in range(KT):
                        nc.tensor.matmul(
                            mp[mt][h],
                            eet_sb[:, ko, ms],
                            we_sb[:, ko, cs],
                            start=(ko == 0),
                            stop=False,
                            skip_group_check=True,
                        )
                    nc.tensor.matmul(
                        mp[mt][h],
                        ones_big[yp_row : yp_row + 1, :P],
                        y_bf[yp_row : yp_row + 1, cs],
                        start=False, stop=True, skip_group_check=True,
                    )
                    if h == 0:
                        nc.vector.tensor_copy(o[:, cs], mp[mt][h])
                    else:
                        nc.scalar.copy(o[:, cs], mp[mt][h])
                    nc.sync.dma_start(out[mt, :, cs], o[:, cs])

    nc.compile()
    _CACHE["nc"] = nc
    return nc


def run(inputs, trace=False, **kwargs):
    nc = _build()
    x = np.asarray(inputs["encoded_utterance"], np.float32)
    ee = np.asarray(inputs["element_embeddings"], np.float32)
    w = np.asarray(inputs["weight_matrix"], np.float32)
    wp = np.asarray(inputs["W_proj"], np.float32)
    bp = np.asarray(inputs["b_proj"], np.float32).reshape(1, D)

    # shared weight packs (k-slice major, bf16)
    wu_p = np.ascontiguousarray(w[:D].reshape(KT, P, C)).astype(BF)
    we_p = np.ascontiguousarray(w[D:].reshape(KT, P, C)).astype(BF)
    wp_p = np.ascontiguousarray(wp.reshape(KT, P, C)).astype(BF)
    bp_p = bp.astype(BF)

    in_maps = []
    for i in range(NCORES):
        bs = slice(i * BPC, (i + 1) * BPC)
        # eeT: [4, 256, D] -> [m=1024, D] -> [D, m] -> [KT, P, m]
        ee_c = ee[bs].reshape(BPC * N, D)
        eet_p = np.ascontiguousarray(ee_c.T.reshape(KT, P, MT * P)).astype(BF)
        # xT: [4, D] -> [D, 4] -> [KT, P, 4] -> [P, KT, 4]
        xt_p = np.ascontiguousarray(
            x[bs].T.reshape(KT, P, BPC).transpose(1, 0, 2)
        ).astype(BF)
        in_maps.append(
            {
                "eet": eet_p,
                "we": we_p,
                "wp": wp_p,
                "wu": wu_p,
                "xt": xt_p,
                "bp": bp_p,
            }
        )

    res = run_bass_kernel_spmd(
        nc, in_maps, core_ids=list(range(NCORES)), trace=trace, **kwargs
    )
    full = np.concatenate(
        [
            np.asarray(r["logits"]).astype(np.float32).reshape(BPC, N, C)
            for r in res.results
        ],
        axis=0,
    )
    return full, res


def kernel(**inputs) -> np.ndarray:
    return run(inputs, trace=False)[0]

